# revision 12
# baseline (speedup 1.0000x reference)
"""Trainium2 Bass kernel for nn_FFDGenerator: cubic B-spline free-form deformation.

Computes flow[b,c,x,y,z] = sum_{i,j,k} Wx[x,i]*Wy[y,j]*Wz[z,k]*mesh[b,c,i,j,k]
where Wx/Wy/Wz are dense per-axis cubic B-spline weight matrices (4 nonzeros
per row, spacing 8), mesh is (4,3,23,27,23), flow is (4,3,160,192,160).

Sharding: output x-axis split into 8 chunks of 20, one per NeuronCore.
Control-point mesh is replicated (bc-triples spread over partition bands).

Per-core pipeline (all contractions on the tensor engine, fp16 in / fp32 PSUM):
  warmup: ~3.5us of dummy matmuls so the PE HAM clock-gate opens (K=8/8)
  before the real work starts; the whole kernel then runs at 2.4 GHz.
  MM1: contract i -> A[xl, (bc,j,k)]       col-tiled over 4 partition bands
  T:   DVE 32x32 block transpose -> A_t[k, (bc,xl,j)] per band, fp16 copy
  MM2: contract k -> C[(xl4,j32), (g,z)]   row-banded, data-stationary
  MM3: contract j -> out[(xl4,y32), z]     block-diag Wy weights, M=128
  Epilogue: per supertile, two 3-bank PSUM tiles evacuated with ONE fp16
  copy each (ACT gets one + the C copy, DVE the other), then one 736KB DMA
  per supertile of 12 output slices, alternating sync/gpsimd queues.

Numerics: single fp16 everywhere (weights, mesh, A, C, staged output) with
fp32 PSUM accumulation; output is stored as fp16 and upcast to fp32 on the
host. End-to-end rel err ~7e-4 vs the 2e-2 gate.
"""

import numpy as np

import concourse.bass as bass
import concourse.mybir as mybir
from concourse.tile import TileContext
from concourse.bass_utils import run_bass_kernel_spmd

F16 = mybir.dt.float16
F32 = mybir.dt.float32

NCORES = 8
B, C = 4, 3
BC = B * C                    # 12 bc slices
X, Y, Z = 160, 192, 160
XL = X // NCORES              # 20 x per core
CX, CY, CZ = 23, 27, 23       # control points per axis
J32 = 32                      # padded j
K32 = 32                      # padded k
NB = 4                        # partition bands
BAND_BC = 3                   # bc per band
FREE1 = BAND_BC * J32 * K32   # 3072: per-band free size of meshT/A
NGRP = BC * XL // 4           # 60 groups of 4 (bc,x) slices
NYC = Y // 32                 # 6 y-chunks
GG = 3                        # groups per supertile
NGP = NGRP // GG              # 20 supertiles
CHUNK = 512                   # MM1 free chunk (one PSUM bank of fp32)
NCH = FREE1 // CHUNK          # 6 chunks
NWARM = 7                     # PE warmup matmuls (~3.5us at cold clock)

_cache = {}


def _axis_w(n, sp, ncp):
    """Dense [n, ncp] cubic B-spline weight matrix (float64)."""
    v = np.arange(n, dtype=np.float64) / sp
    f = np.floor(v)
    d = v - f
    w = np.stack(
        [
            (1 - d) ** 3 / 6,
            d**3 / 2 - d**2 + 2.0 / 3,
            -(d**3) / 2 + d**2 / 2 + d / 2 + 1.0 / 6,
            d**3 / 6,
        ],
        -1,
    )
    W = np.zeros((n, ncp))
    idx = f.astype(int)
    for a in range(4):
        W[np.arange(n), idx + a] = w[:, a]
    return W


def _host_weights():
    if "w" in _cache:
        return _cache["w"]
    Wx = _axis_w(X, 8, CX)
    Wy = _axis_w(Y, 8, CY)
    Wz = _axis_w(Z, 8, CZ)

    # wx3[core]: [128, 32] fp16, WxT chunk replicated on 4 partition bands
    wx3 = np.zeros((NCORES, 128, 32), np.float16)
    for core in range(NCORES):
        blk = Wx[core * XL : (core + 1) * XL, :].T.astype(np.float16)  # [23, 20]
        for q in range(NB):
            wx3[core, 32 * q : 32 * q + CX, :XL] = blk

    # wz3: [128, Z] fp16, WzT replicated on 4 partition bands
    wz3 = np.zeros((128, Z), np.float16)
    for q in range(NB):
        wz3[32 * q : 32 * q + CZ, :] = Wz.T.astype(np.float16)

    # wyb: [128, NYC*128] fp16 block-diag. Variant c covers the strided y-set
    # y = 6*yi + c (yi = 0..31): wyb[32b+j, 128c + 32b+yi] = Wy[6yi+c, j].
    # The stride-6 y interleave makes each slice's (yi, c, z) staging layout
    # land contiguously in flow[y, z], so one DMA stores a whole slice quad.
    wyb = np.zeros((128, NYC * 128), np.float16)
    for c in range(NYC):
        for b in range(4):
            for j in range(CY):
                wyb[32 * b + j, 128 * c + 32 * b : 128 * c + 32 * b + 32] = Wy[
                    c : c + 6 * 32 : 6, j
                ].astype(np.float16)

    _cache["w"] = (wx3, wz3, wyb)
    return _cache["w"]


def _prep_mesh(mesh):
    """mesh [4,3,23,27,23] f32 -> meshT [128, FREE1] fp16.

    Partition 32q+i holds mesh[bc=3q+bcq, i, j, k] at free index
    bcq*J32*K32 + j*K32 + k (j, k zero-padded to 32).
    """
    m = np.asarray(mesh, np.float32).reshape(BC, CX, CY, CZ)
    mt = np.zeros((128, BAND_BC, J32, K32), np.float32)
    for q in range(NB):
        for bcq in range(BAND_BC):
            bc = 3 * q + bcq
            mt[32 * q : 32 * q + CX, bcq, :CY, :CZ] = m[bc]
    return mt.astype(np.float16).reshape(128, FREE1)


def _build_program():
    if "nc" in _cache:
        return _cache["nc"]
    nc = bass.Bass()
    mesh16 = nc.declare_dram_parameter("mesh16", [128, FREE1], F16, isOutput=False)
    wx3 = nc.declare_dram_parameter("wx3", [128, 32], F16, isOutput=False)
    wz3 = nc.declare_dram_parameter("wz3", [128, Z], F16, isOutput=False)
    wyb = nc.declare_dram_parameter("wyb", [128, NYC * 128], F16, isOutput=False)
    flow = nc.declare_dram_parameter("flow", [BC * XL, Y, Z], F16, isOutput=True)

    # Store view: slice s = 12*gp + 4*gg + b, y = 6*yi + c. With staging laid
    # out [(b,yi) partitions, (gg,c,z) free], a whole supertile (12 slices,
    # 3x245KB contiguous quads) is one DMA:
    # dst offset = 960*(32b+yi) + 160c + z  (elements, relative to quad base).
    flowV = flow[:, :, :].rearrange(
        "(gp gg b) (yi c) z -> gp (b yi) gg (c z)", gg=GG, b=4, c=NYC
    )

    with TileContext(nc) as tc:
        with (
            tc.tile_pool(name="const", bufs=1) as cpool,
            tc.tile_pool(name="abuf", bufs=1) as apool,
            tc.tile_pool(name="cbuf", bufs=3) as cbpool,
            tc.tile_pool(name="stage", bufs=4) as spool,
            tc.tile_pool(name="psA", bufs=1, space="PSUM") as psApool,
            tc.tile_pool(name="ps3", bufs=2, space="PSUM") as ps3pool,
        ):
            # Inputs. wx first on the fast HWDGE queue so MM1 can start early;
            # mesh chunk ch gets its own tile so MM1 ch starts as soon as its
            # own slice of the mesh has landed (Tile deps are whole-tile).
            wx = cpool.tile([128, 32], F16, tag="wx")
            nc.sync.dma_start(out=wx[:, :], in_=wx3[:, :])
            m2c = [
                cpool.tile([128, CHUNK], F16, name=f"m2{ch}", tag=f"m2{ch}")
                for ch in range(NCH)
            ]
            wz = cpool.tile([128, Z], F16, tag="wz")
            wy = cpool.tile([128, NYC * 128], F16, tag="wy")
            nc.gpsimd.dma_start(out=m2c[1][:, :], in_=mesh16[:, CHUNK : 2 * CHUNK])
            nc.sync.dma_start(out=m2c[0][:, :], in_=mesh16[:, :CHUNK])
            nc.gpsimd.dma_start(out=wz[:, :], in_=wz3[:, :])
            nc.sync.dma_start(out=wy[:, :], in_=wyb[:, :])
            for ch in range(2, NCH):
                s = slice(ch * CHUNK, (ch + 1) * CHUNK)
                eng = nc.sync if ch % 2 == 0 else nc.gpsimd
                eng.dma_start(out=m2c[ch][:, :], in_=mesh16[:, s])

            # PE warmup + fillers: the HAM clock gate needs ~3.4us of
            # sustained PE activity to open (K=8/8 -> 2.4 GHz) and re-closes
            # whenever PE occupancy drops for a ~3.4us window. The real work
            # only keeps PE ~60% busy (the loop is store-DMA paced), which is
            # not enough to hold the gate open, so we weave dummy matmuls
            # through the whole kernel. They MUST chew real (nonzero) data:
            # the activity monitor watches MAC switching, and an all-zeros
            # matmul is invisible to it. They write a PSUM slot nobody reads
            # (p1, free after MM1) and cost ~0.2us each once warm.
            def filler(n=1):
                for _ in range(n):
                    pw = psApool.tile([128, 512], F32, tag="p1", name="p1")
                    nc.tensor.matmul(
                        pw[:, :],
                        lhsT=m2c[0][:, :128],
                        rhs=m2c[0][:, :],
                        start=True,
                        stop=True,
                    )

            filler(NWARM)

            # ---- MM1 (contract i) + 32x32 block transpose ----
            # Separate A tiles per bc-triple (bcq) so MM2 groups that consume
            # one triple can start while later chunks are still in MM1.
            PB = J32 * K32  # 1024: per-bcq free size
            at = [apool.tile([128, PB], F32, name=f"at{b}", tag=f"at{b}")
                  for b in range(BAND_BC)]
            # fp16 A, stored permuted (j,x)->(x,j) so each MM2 lhsT is one
            # contiguous 128-wide run (walrus: 1 free dim).
            ah = [apool.tile([128, PB], F16, name=f"ah{b}", tag=f"ah{b}")
                  for b in range(BAND_BC)]
            for ch in range(NCH):
                p1 = psApool.tile([128, CHUNK], F32, tag="p1", name="p1")
                for q in range(NB):
                    band = slice(32 * q, 32 * q + CX)
                    nc.tensor.matmul(
                        p1[32 * q : 32 * q + 32, :],
                        lhsT=wx[band, :],
                        rhs=m2c[ch][band, :],
                        start=True,
                        stop=True,
                        tile_position=(32 * q, 32 * q),
                    )
                bq, half = ch // 2, (ch % 2) * CHUNK
                nc.vector.transpose(
                    out=at[bq][:, half : half + CHUNK], in_=p1[:, :]
                )
                filler(2)  # MM1 is transpose-paced; keep the PE gate open
                # fp16 downcast of the freshly transposed j-half (j-halves
                # land interleaved in ah's (x, j) layout) so ah[bq] is
                # complete right after the second transpose of the triple.
                h = ch % 2
                atP = at[bq][:, :].rearrange("p (j x) -> p x j", j=J32, x=K32)
                ahV = ah[bq][:, :].rearrange("p (x j) -> p x j", x=K32, j=J32)
                nc.scalar.copy(
                    out=ahV[:, :, 16 * h : 16 * h + 16],
                    in_=atP[:, :, 16 * h : 16 * h + 16],
                )

            # ---- MM2 (contract k) + C copy + MM3 (contract j) + store ----
            # Fully interleaved per gp (= one supertile of 3 slice quads):
            # MM2 produces C for this gp, MM3 consumes it immediately; the
            # fp16 staging tile is stored with ONE DMA (736KB of flow).
            def emit_mm2(gp):
                """Produce C (fp16) for supertile gp."""
                p2 = psApool.tile([128, GG * Z], F32, tag="p2", name="p2")
                for sub in range(GG):
                    g = gp * GG + sub
                    bc = g // 5
                    q, bq, xg = bc // 3, bc % 3, g % 5
                    lo = 128 * xg
                    nc.tensor.matmul(
                        p2[:, sub * Z : (sub + 1) * Z],
                        lhsT=ah[bq][32 * q : 32 * q + CZ, lo : lo + 128],
                        rhs=wz[32 * q : 32 * q + CZ, :],
                        start=True,
                        stop=True,
                        tile_position=(32 * q, 0),
                    )
                chl = cbpool.tile([128, GG * Z], F16, name="chl", tag="chl")
                if gp % 2 == 0:
                    nc.scalar.copy(out=chl[:, :], in_=p2[:, :])
                else:
                    nc.vector.tensor_copy(chl[:, :], p2[:, :])
                return chl

            def emit_mm3(gp, chl):
                """Contract j for supertile gp, stage fp16, one-DMA store."""
                # stg free layout: (gg, c, z) so the store DMA reads 1920B
                # contiguous per (partition, gg) and flow dst is contiguous.
                stg = spool.tile([128, GG * NYC * Z], F16, tag="stg", name="stg")
                stgD = stg[:, :].rearrange(
                    "p (gg c z) -> p c gg z", gg=GG, c=NYC, z=Z
                )
                for half in range(2):  # y-variant triples (c = 3*half + cc)
                    # three bank-aligned [128,480] outputs in a 3-bank tile
                    p3 = ps3pool.tile([128, 1536], F32, tag="p3", name="p3")
                    for cc in range(3):
                        c = 3 * half + cc
                        nc.tensor.matmul(
                            p3[:, cc * 512 : cc * 512 + GG * Z],
                            lhsT=wy[:, 128 * c : 128 * (c + 1)],
                            rhs=chl[:, :],
                            start=True,
                            stop=True,
                        )
                    # evac: src (cc, gg, z) -> dst (c, gg, z); one strided
                    # copy per half moves 1440 elements (skips bank padding).
                    a = p3[:, :]
                    src = bass.AP(
                        a.tensor, a.offset,
                        [a.ap[0], [512, 3], [Z, GG], [1, Z]],
                    )
                    dst = stgD[:, 3 * half : 3 * half + 3]
                    if half == 0:
                        nc.scalar.copy(out=dst, in_=src)
                    else:
                        nc.vector.tensor_copy(dst, src)
                eng = nc.sync if gp % 2 == 0 else nc.gpsimd
                eng.dma_start(out=flowV[gp], in_=stg[:, :].rearrange(
                    "p (gg cz) -> p gg cz", gg=GG
                ))
                filler(2 if gp < 8 else 1)  # hold the PE clock gate open

            # Software pipeline: depth 1 while filling (early gps depend on
            # freshly-transposed A triples; shallow depth gets the first
            # store out ~2us sooner), depth 2 in steady state.
            pend = []
            for gp in range(NGP):
                pend.append((gp, emit_mm2(gp)))
                if len(pend) > (1 if gp < 3 else 2):
                    emit_mm3(*pend.pop(0))
            for item in pend:
                emit_mm3(*item)

    # Walrus allows at most one sync-wait per matmul; split extras into
    # EventSemaphore instructions (same pass Bacc.compile runs).
    import bass_rust as _bass_rust

    _bass_rust.move_matmul_waits_to_ldweights(nc.m)
    _bass_rust.generate_event_semaphores(nc)

    _cache["nc"] = nc
    return nc


def _in_maps(mesh):
    wx3, wz3, wyb = _host_weights()
    mh = _prep_mesh(mesh)
    return [
        {"mesh16": mh, "wx3": wx3[core], "wz3": wz3, "wyb": wyb}
        for core in range(NCORES)
    ]


def kernel(mesh: np.ndarray) -> np.ndarray:
    nc = _build_program()
    in_maps = _in_maps(mesh)
    last_err = None
    for attempt in range(3):
        try:
            res = run_bass_kernel_spmd(nc, in_maps, list(range(NCORES))).results
            break
        except Exception as e:  # transient device wedge: retry
            last_err = e
    else:
        raise last_err
    full = np.empty((BC, X, Y, Z), np.float32)
    for core in range(NCORES):
        full[:, core * XL : (core + 1) * XL] = res[core]["flow"].reshape(
            BC, XL, Y, Z
        )
    return full.reshape(B, C, X, Y, Z)


# revision 13
# speedup vs baseline: 1.3295x; 1.3295x over previous
"""Trainium2 Bass kernel for nn_FFDGenerator: cubic B-spline free-form deformation.

Computes flow[b,c,x,y,z] = sum_{i,j,k} Wx[x,i]*Wy[y,j]*Wz[z,k]*mesh[b,c,i,j,k]
where Wx/Wy/Wz are dense per-axis cubic B-spline weight matrices (4 nonzeros
per row, spacing 8), mesh is (4,3,23,27,23), flow is (4,3,160,192,160).

Sharding: output x-axis split into 8 chunks of 20, one per NeuronCore.
Control-point mesh is replicated (bc-triples spread over partition bands).

Per-core pipeline (all contractions on the tensor engine, fp16 in / fp32 PSUM):
  MM1: contract i -> A[xl, (bc,j,k)]       col-tiled over 4 partition bands
  T:   DVE 32x32 block transpose -> A_t[k, (bc,xl,j)] per band, fp16 copy
  MM2: contract k -> C[(xl4,j32), (g,z)]   row-banded, data-stationary
  MM3: contract j -> out[(xl4,y32), z]     block-diag Wy weights, M=128
  Epilogue: per supertile, three 2-bank PSUM tiles (triple-buffered)
  evacuated with one fp16 copy each (ACT takes 2, DVE takes 1 + the C
  copy), then one 736KB DMA per supertile of 12 output slices,
  alternating sync/gpsimd queues.

Numerics: single fp16 everywhere (weights, mesh, A, C, staged output) with
fp32 PSUM accumulation; output is stored as fp16 and upcast to fp32 on the
host. End-to-end rel err ~7e-4 vs the 2e-2 gate.
"""

import numpy as np

import concourse.bass as bass
import concourse.mybir as mybir
from concourse.tile import TileContext
from concourse.bass_utils import run_bass_kernel_spmd

F16 = mybir.dt.float16
F32 = mybir.dt.float32

NCORES = 8
B, C = 4, 3
BC = B * C                    # 12 bc slices
X, Y, Z = 160, 192, 160
XL = X // NCORES              # 20 x per core
CX, CY, CZ = 23, 27, 23       # control points per axis
J32 = 32                      # padded j
K32 = 32                      # padded k
NB = 4                        # partition bands
BAND_BC = 3                   # bc per band
FREE1 = BAND_BC * J32 * K32   # 3072: per-band free size of meshT/A
NGRP = BC * XL // 4           # 60 groups of 4 (bc,x) slices
NYC = Y // 32                 # 6 y-chunks
GG = 3                        # groups per supertile
NGP = NGRP // GG              # 20 supertiles
CHUNK = 512                   # MM1 free chunk (one PSUM bank of fp32)
NCH = FREE1 // CHUNK          # 6 chunks

_cache = {}


def _axis_w(n, sp, ncp):
    """Dense [n, ncp] cubic B-spline weight matrix (float64)."""
    v = np.arange(n, dtype=np.float64) / sp
    f = np.floor(v)
    d = v - f
    w = np.stack(
        [
            (1 - d) ** 3 / 6,
            d**3 / 2 - d**2 + 2.0 / 3,
            -(d**3) / 2 + d**2 / 2 + d / 2 + 1.0 / 6,
            d**3 / 6,
        ],
        -1,
    )
    W = np.zeros((n, ncp))
    idx = f.astype(int)
    for a in range(4):
        W[np.arange(n), idx + a] = w[:, a]
    return W


def _host_weights():
    if "w" in _cache:
        return _cache["w"]
    Wx = _axis_w(X, 8, CX)
    Wy = _axis_w(Y, 8, CY)
    Wz = _axis_w(Z, 8, CZ)

    # wx3[core]: [128, 32] fp16, WxT chunk replicated on 4 partition bands
    wx3 = np.zeros((NCORES, 128, 32), np.float16)
    for core in range(NCORES):
        blk = Wx[core * XL : (core + 1) * XL, :].T.astype(np.float16)  # [23, 20]
        for q in range(NB):
            wx3[core, 32 * q : 32 * q + CX, :XL] = blk

    # wz3: [128, Z] fp16, WzT replicated on 4 partition bands
    wz3 = np.zeros((128, Z), np.float16)
    for q in range(NB):
        wz3[32 * q : 32 * q + CZ, :] = Wz.T.astype(np.float16)

    # wyb: [128, NYC*128] fp16 block-diag. Variant c covers the strided y-set
    # y = 6*yi + c (yi = 0..31): wyb[32b+j, 128c + 32b+yi] = Wy[6yi+c, j].
    # The stride-6 y interleave makes each slice's (yi, c, z) staging layout
    # land contiguously in flow[y, z], so one DMA stores a whole slice quad.
    wyb = np.zeros((128, NYC * 128), np.float16)
    for c in range(NYC):
        for b in range(4):
            for j in range(CY):
                wyb[32 * b + j, 128 * c + 32 * b : 128 * c + 32 * b + 32] = Wy[
                    c : c + 6 * 32 : 6, j
                ].astype(np.float16)

    _cache["w"] = (wx3, wz3, wyb)
    return _cache["w"]


def _prep_mesh(mesh):
    """mesh [4,3,23,27,23] f32 -> meshT [128, FREE1] fp16.

    Partition 32q+i holds mesh[bc=3q+bcq, i, j, k] at free index
    bcq*J32*K32 + j*K32 + k (j, k zero-padded to 32).
    """
    m = np.asarray(mesh, np.float32).reshape(BC, CX, CY, CZ)
    mt = np.zeros((128, BAND_BC, J32, K32), np.float32)
    for q in range(NB):
        for bcq in range(BAND_BC):
            bc = 3 * q + bcq
            mt[32 * q : 32 * q + CX, bcq, :CY, :CZ] = m[bc]
    return mt.astype(np.float16).reshape(128, FREE1)


def _build_program():
    if "nc" in _cache:
        return _cache["nc"]
    nc = bass.Bass()
    mesh16 = nc.declare_dram_parameter("mesh16", [128, FREE1], F16, isOutput=False)
    wx3 = nc.declare_dram_parameter("wx3", [128, 32], F16, isOutput=False)
    wz3 = nc.declare_dram_parameter("wz3", [128, Z], F16, isOutput=False)
    wyb = nc.declare_dram_parameter("wyb", [128, NYC * 128], F16, isOutput=False)
    flow = nc.declare_dram_parameter("flow", [BC * XL, Y, Z], F16, isOutput=True)

    # Store view: slice s = 12*gp + 4*gg + b, y = 6*yi + c. With staging laid
    # out [(b,yi) partitions, (gg,c,z) free], a whole supertile (12 slices,
    # 3x245KB contiguous quads) is one DMA:
    # dst offset = 960*(32b+yi) + 160c + z  (elements, relative to quad base).
    flowV = flow[:, :, :].rearrange(
        "(gp gg b) (yi c) z -> gp (b yi) gg (c z)", gg=GG, b=4, c=NYC
    )

    with TileContext(nc) as tc:
        with (
            tc.tile_pool(name="const", bufs=1) as cpool,
            tc.tile_pool(name="abuf", bufs=1) as apool,
            tc.tile_pool(name="cbuf", bufs=3) as cbpool,
            tc.tile_pool(name="stage", bufs=4) as spool,
            tc.tile_pool(name="psA", bufs=2, space="PSUM") as psApool,
            tc.tile_pool(name="ps3", bufs=3, space="PSUM") as ps3pool,
        ):
            # Inputs. wx first on the fast HWDGE queue so MM1 can start early;
            # mesh chunk ch gets its own tile so MM1 ch starts as soon as its
            # own slice of the mesh has landed (Tile deps are whole-tile).
            wx = cpool.tile([128, 32], F16, tag="wx")
            nc.sync.dma_start(out=wx[:, :], in_=wx3[:, :])
            m2c = [
                cpool.tile([128, CHUNK], F16, name=f"m2{ch}", tag=f"m2{ch}")
                for ch in range(NCH)
            ]
            wz = cpool.tile([128, Z], F16, tag="wz")
            wy = cpool.tile([128, NYC * 128], F16, tag="wy")
            nc.gpsimd.dma_start(out=m2c[1][:, :], in_=mesh16[:, CHUNK : 2 * CHUNK])
            nc.sync.dma_start(out=m2c[0][:, :], in_=mesh16[:, :CHUNK])
            nc.gpsimd.dma_start(out=wz[:, :], in_=wz3[:, :])
            nc.sync.dma_start(out=wy[:, :], in_=wyb[:, :])
            for ch in range(2, NCH):
                s = slice(ch * CHUNK, (ch + 1) * CHUNK)
                eng = nc.sync if ch % 2 == 0 else nc.gpsimd
                eng.dma_start(out=m2c[ch][:, :], in_=mesh16[:, s])


            # ---- MM1 (contract i) + 32x32 block transpose ----
            # Separate A tiles per bc-triple (bcq) so MM2 groups that consume
            # one triple can start while later chunks are still in MM1.
            PB = J32 * K32  # 1024: per-bcq free size
            at = [apool.tile([128, PB], F32, name=f"at{b}", tag=f"at{b}")
                  for b in range(BAND_BC)]
            # fp16 A, stored permuted (j,x)->(x,j) so each MM2 lhsT is one
            # contiguous 128-wide run (walrus: 1 free dim).
            ah = [apool.tile([128, PB], F16, name=f"ah{b}", tag=f"ah{b}")
                  for b in range(BAND_BC)]
            for ch in range(NCH):
                p1 = psApool.tile([128, CHUNK], F32, tag="p12", name="p12")
                for q in range(NB):
                    band = slice(32 * q, 32 * q + CX)
                    nc.tensor.matmul(
                        p1[32 * q : 32 * q + 32, :],
                        lhsT=wx[band, :],
                        rhs=m2c[ch][band, :],
                        start=True,
                        stop=True,
                        tile_position=(32 * q, 32 * q),
                    )
                bq, half = ch // 2, (ch % 2) * CHUNK
                nc.vector.transpose(
                    out=at[bq][:, half : half + CHUNK], in_=p1[:, :]
                )
                # fp16 downcast of the freshly transposed j-half (j-halves
                # land interleaved in ah's (x, j) layout) so ah[bq] is
                # complete right after the second transpose of the triple.
                h = ch % 2
                atP = at[bq][:, :].rearrange("p (j x) -> p x j", j=J32, x=K32)
                ahV = ah[bq][:, :].rearrange("p (x j) -> p x j", x=K32, j=J32)
                nc.scalar.copy(
                    out=ahV[:, :, 16 * h : 16 * h + 16],
                    in_=atP[:, :, 16 * h : 16 * h + 16],
                )

            # ---- MM2 (contract k) + C copy + MM3 (contract j) + store ----
            # Fully interleaved per gp (= one supertile of 3 slice quads):
            # MM2 produces C for this gp, MM3 consumes it immediately; the
            # fp16 staging tile is stored with ONE DMA (736KB of flow).
            def emit_mm2(gp):
                """Produce C (fp16) for supertile gp."""
                p2 = psApool.tile([128, CHUNK], F32, tag="p12", name="p12")
                for sub in range(GG):
                    g = gp * GG + sub
                    bc = g // 5
                    q, bq, xg = bc // 3, bc % 3, g % 5
                    lo = 128 * xg
                    nc.tensor.matmul(
                        p2[:, sub * Z : (sub + 1) * Z],
                        lhsT=ah[bq][32 * q : 32 * q + CZ, lo : lo + 128],
                        rhs=wz[32 * q : 32 * q + CZ, :],
                        start=True,
                        stop=True,
                        tile_position=(32 * q, 0),
                    )
                chl = cbpool.tile([128, GG * Z], F16, name="chl", tag="chl")
                nc.vector.tensor_copy(chl[:, :], p2[:, : GG * Z])
                return chl

            def emit_mm3(gp, chl):
                """Contract j for supertile gp, stage fp16, one-DMA store."""
                # stg free layout: (gg, c, z) so the store DMA reads 1920B
                # contiguous per (partition, gg) and flow dst is contiguous.
                stg = spool.tile([128, GG * NYC * Z], F16, tag="stg", name="stg")
                stgD = stg[:, :].rearrange(
                    "p (gg c z) -> p c gg z", gg=GG, c=NYC, z=Z
                )
                for cp in range(NYC // 2):  # y-variant pairs (c = 2*cp + cc)
                    # two bank-aligned [128,480] outputs in a 2-bank tile;
                    # triple-buffered so the PSUM-recycle chain (MM3 of gp
                    # waits evac of gp-1) stays off the critical path.
                    p3 = ps3pool.tile([128, 1024], F32, tag="p3", name="p3")
                    for cc in range(2):
                        c = 2 * cp + cc
                        nc.tensor.matmul(
                            p3[:, cc * 512 : cc * 512 + GG * Z],
                            lhsT=wy[:, 128 * c : 128 * (c + 1)],
                            rhs=chl[:, :],
                            start=True,
                            stop=True,
                        )
                    # evac: src (cc, gg, z) -> dst (c, gg, z); one strided
                    # copy per pair moves 960 elements (skips bank padding).
                    a = p3[:, :]
                    src = bass.AP(
                        a.tensor, a.offset,
                        [a.ap[0], [512, 2], [Z, GG], [1, Z]],
                    )
                    dst = stgD[:, 2 * cp : 2 * cp + 2]
                    if cp == 1:
                        nc.vector.tensor_copy(dst, src)
                    else:
                        nc.scalar.copy(out=dst, in_=src)
                eng = nc.sync if gp % 2 == 0 else nc.gpsimd
                eng.dma_start(out=flowV[gp], in_=stg[:, :].rearrange(
                    "p (gg cz) -> p gg cz", gg=GG
                ))

            # Software pipeline: depth 1 while filling (early gps depend on
            # freshly-transposed A triples; shallow depth gets the first
            # store out ~2us sooner), depth 2 in steady state.
            pend = []
            for gp in range(NGP):
                pend.append((gp, emit_mm2(gp)))
                if len(pend) > (1 if gp < 3 else 2):
                    emit_mm3(*pend.pop(0))
            for item in pend:
                emit_mm3(*item)

    # Walrus allows at most one sync-wait per matmul; split extras into
    # EventSemaphore instructions (same pass Bacc.compile runs).
    import bass_rust as _bass_rust

    _bass_rust.move_matmul_waits_to_ldweights(nc.m)
    _bass_rust.generate_event_semaphores(nc)

    _cache["nc"] = nc
    return nc


def _in_maps(mesh):
    wx3, wz3, wyb = _host_weights()
    mh = _prep_mesh(mesh)
    return [
        {"mesh16": mh, "wx3": wx3[core], "wz3": wz3, "wyb": wyb}
        for core in range(NCORES)
    ]


def kernel(mesh: np.ndarray) -> np.ndarray:
    nc = _build_program()
    in_maps = _in_maps(mesh)
    last_err = None
    for attempt in range(3):
        try:
            res = run_bass_kernel_spmd(nc, in_maps, list(range(NCORES))).results
            break
        except Exception as e:  # transient device wedge: retry
            last_err = e
    else:
        raise last_err
    full = np.empty((BC, X, Y, Z), np.float32)
    for core in range(NCORES):
        full[:, core * XL : (core + 1) * XL] = res[core]["flow"].reshape(
            BC, XL, Y, Z
        )
    return full.reshape(B, C, X, Y, Z)
